# revision 1
# baseline (speedup 1.0000x reference)
"""Tensor-parallel multi-head attention (rotary + masked softmax + output
projection) on 8 TRN2 NeuronCores.

Sharding: 4 heads per core. Each core projects the full token stream onto its
4 heads (Q/K/V), runs attention locally, then the per-head outputs are
AllGathered (bf16, feature-major) and every core computes a disjoint
512-column slice of the final wo projection. The host concatenates slices.

All matmuls run in bf16 (f32 accumulation in PSUM). The attention mask is
treated as data: at call time its 128x128 blocks are classified
(zero / fully-masked / general); fully-masked blocks are skipped entirely
(causal masks drop half the attention work), general blocks are added from
SBUF. The Bass graph is cached per mask block-structure.
"""

import math
import sys

import numpy as np

sys.path.insert(0, "/opt/trn_rl_repo")

import ml_dtypes  # noqa: E402

import concourse.bass as bass  # noqa: E402,F401
import concourse.mybir as mybir  # noqa: E402
from concourse import bacc, tile  # noqa: E402
from concourse.bass_utils import run_bass_kernel_spmd  # noqa: E402
from concourse.masks import make_identity  # noqa: E402

B, S, D, H = 2, 2048, 4096, 32
HD = 128
NCORES = 8
HLOC = H // NCORES          # 4 heads per core
EL = HLOC * HD              # 512 local feature width
T = B * S                   # 4096 tokens
P = 128
NT = T // P                 # 32 token tiles
ND = D // P                 # 32 contraction tiles
NB = S // P                 # 16 query/key blocks per batch element
SCALE = 1.0 / math.sqrt(HD)
BF = mybir.dt.bfloat16
F32 = mybir.dt.float32
TCH = 256                   # token chunk width for x streaming
NCH = T // TCH

_GRAPH_CACHE: dict[bytes, object] = {}
LAST_RESULTS = None  # test harness peeks at this for profiling


def _classify_mask(mask: np.ndarray):
    """Per 128x128 block: -2 all-zero, -1 fully masked (exp -> 0), else an
    index into the deduplicated block table."""
    mb = mask.reshape(NB, P, NB, P).transpose(0, 2, 1, 3)
    kinds = np.empty((NB, NB), np.int64)
    uniq: dict[bytes, int] = {}
    blocks: list[np.ndarray] = []
    for i in range(NB):
        for j in range(NB):
            blk = mb[i, j]
            if not blk.any():
                kinds[i, j] = -2
            elif np.all(blk <= -1e8):
                kinds[i, j] = -1
            else:
                key = blk.tobytes()
                if key not in uniq:
                    uniq[key] = len(blocks)
                    blocks.append(np.ascontiguousarray(blk, np.float32))
                kinds[i, j] = uniq[key]
    return kinds, blocks


def _pieces(jlist):
    """Split jlist (sorted block columns) into maximal consecutive runs of at
    most 4 blocks -> one <=512-wide matmul per piece."""
    out = []
    run = [jlist[0]]
    for j in jlist[1:]:
        if j == run[-1] + 1 and len(run) < 4:
            run.append(j)
        else:
            out.append(run)
            run = [j]
    out.append(run)
    return out


def _build(kinds: np.ndarray, n_blocks: int):
    nu = max(1, n_blocks)
    nc = bacc.Bacc("TRN2", target_bir_lowering=False, debug=False,
                   num_devices=NCORES)
    xt = nc.dram_tensor("xt", [D, T], BF, kind="ExternalInput")
    wqt = nc.dram_tensor("wqt", [D, EL], BF, kind="ExternalInput")
    wkt = nc.dram_tensor("wkt", [D, EL], BF, kind="ExternalInput")
    wvt = nc.dram_tensor("wvt", [D, EL], BF, kind="ExternalInput")
    wot = nc.dram_tensor("wot", [D, EL], BF, kind="ExternalInput")
    cosr = nc.dram_tensor("cosr", [S, EL // 2], F32, kind="ExternalInput")
    sinr = nc.dram_tensor("sinr", [S, EL // 2], F32, kind="ExternalInput")
    mblk = nc.dram_tensor("mblk", [nu, P, P], F32, kind="ExternalInput")
    out = nc.dram_tensor("out", [T, EL], F32, kind="ExternalOutput")

    qtd = nc.dram_tensor("qtd", [EL, T], BF)      # Q^T (feature, token)
    ktd = nc.dram_tensor("ktd", [EL, T], BF)
    vd = nc.dram_tensor("vd", [T, EL], BF)        # V natural (token, feature)
    cc_in = nc.dram_tensor("cc_in", [EL, T], BF)
    cc_out = nc.dram_tensor("cc_out", [NCORES * EL, T], BF,
                            addr_space="Shared")

    with tile.TileContext(nc) as tc:
        with tc.tile_pool(name="const", bufs=1) as cpool:
            ident = cpool.tile([P, P], BF, name="ident")
            make_identity(nc, ident)
            mb_sb = cpool.tile([P, nu, P], F32, name="mb_sb")
            nc.sync.dma_start(mb_sb, mblk.ap().rearrange("n p q -> p n q"))

            # ---------------- projection phase: Q/K/V = x @ w^T ----------
            with (
                tc.tile_pool(name="wpool", bufs=1) as wpool,
                tc.tile_pool(name="xpool", bufs=2) as xpool,
                tc.tile_pool(name="cspool", bufs=2) as cspool,
                tc.tile_pool(name="stg", bufs=3) as stg,
                tc.tile_pool(name="rot", bufs=2) as rot,
                tc.tile_pool(name="pp", bufs=3, space="PSUM") as pp,
                tc.tile_pool(name="tp", bufs=3, space="PSUM") as tp,
            ):
                wq_sb = wpool.tile([P, ND, EL], BF, name="wq_sb")
                wk_sb = wpool.tile([P, ND, EL], BF, name="wk_sb")
                wv_sb = wpool.tile([P, ND, EL], BF, name="wv_sb")
                for w_sb, w_d in ((wq_sb, wqt), (wk_sb, wkt), (wv_sb, wvt)):
                    nc.sync.dma_start(
                        w_sb, w_d.ap().rearrange("(n p) e -> p n e", p=P))

                for ch in range(NCH):
                    xt_sb = xpool.tile([P, ND, TCH], BF, tag="xt")
                    nc.sync.dma_start(
                        xt_sb,
                        xt.ap()[:, ch * TCH:(ch + 1) * TCH]
                        .rearrange("(n p) t -> p n t", p=P))
                    for tt in range(TCH // P):
                        g = ch * (TCH // P) + tt
                        s0 = (g * P) % S
                        cs_sb = cspool.tile([P, EL // 2], F32, tag="cs")
                        sn_sb = cspool.tile([P, EL // 2], F32, tag="sn")
                        nc.sync.dma_start(cs_sb, cosr.ap()[s0:s0 + P, :])
                        nc.sync.dma_start(sn_sb, sinr.ap()[s0:s0 + P, :])
                        lhs = xt_sb[:, :, tt * P:(tt + 1) * P]

                        ps_v = pp.tile([P, EL], F32, tag="psqkv")
                        for dt in range(ND):
                            nc.tensor.matmul(ps_v, lhs[:, dt], wv_sb[:, dt],
                                             start=(dt == 0),
                                             stop=(dt == ND - 1))
                        v_stage = stg.tile([P, EL], BF, tag="vstg")
                        nc.any.tensor_copy(v_stage, ps_v)
                        nc.sync.dma_start(vd.ap()[g * P:(g + 1) * P, :],
                                          v_stage)

                        for w_sb, dstd in ((wq_sb, qtd), (wk_sb, ktd)):
                            ps_q = pp.tile([P, EL], F32, tag="psqkv")
                            for dt in range(ND):
                                nc.tensor.matmul(ps_q, lhs[:, dt],
                                                 w_sb[:, dt],
                                                 start=(dt == 0),
                                                 stop=(dt == ND - 1))
                            qn = stg.tile([P, EL], BF, tag="qn")
                            pe = ps_q.rearrange("p (r two) -> p r two", two=2)
                            qe = qn.rearrange("p (r two) -> p r two", two=2)
                            t1 = rot.tile([P, EL // 2], F32, tag="t1")
                            t2 = rot.tile([P, EL // 2], F32, tag="t2")
                            nc.vector.tensor_mul(t1, pe[:, :, 0], cs_sb)
                            nc.vector.tensor_mul(t2, pe[:, :, 1], sn_sb)
                            nc.vector.tensor_sub(qe[:, :, 0], t1, t2)
                            nc.vector.tensor_mul(t1, pe[:, :, 0], sn_sb)
                            nc.vector.tensor_mul(t2, pe[:, :, 1], cs_sb)
                            nc.vector.tensor_add(qe[:, :, 1], t1, t2)
                            for hh in range(HLOC):
                                tps = tp.tile([P, P], BF, tag="tps")
                                nc.tensor.transpose(
                                    tps, qn[:, hh * P:(hh + 1) * P], ident)
                                qtb = stg.tile([P, P], BF, tag="qtb")
                                nc.any.tensor_copy(qtb, tps)
                                nc.sync.dma_start(
                                    dstd.ap()[hh * P:(hh + 1) * P,
                                              g * P:(g + 1) * P], qtb)

            # ---------------- attention phase ---------------------------
            with (
                tc.tile_pool(name="apool", bufs=2) as apool,
                tc.tile_pool(name="ppool", bufs=2) as ppool,
                tc.tile_pool(name="ptpool", bufs=3) as ptpool,
                tc.tile_pool(name="dpool", bufs=2) as dpool,
                tc.tile_pool(name="opool", bufs=2) as opool,
                tc.tile_pool(name="spp", bufs=2, space="PSUM") as spp,
                tc.tile_pool(name="ptp", bufs=3, space="PSUM") as ptp,
                tc.tile_pool(name="otp", bufs=2, space="PSUM") as otp,
            ):
                for b in range(B):
                    for hh in range(HLOC):
                        qt_i = apool.tile([P, S], BF, tag="qt_i")
                        kt_i = apool.tile([P, S], BF, tag="kt_i")
                        v_i = apool.tile([P, NB, P], BF, tag="v_i")
                        nc.sync.dma_start(
                            qt_i,
                            qtd.ap()[hh * P:(hh + 1) * P, b * S:(b + 1) * S])
                        nc.sync.dma_start(
                            kt_i,
                            ktd.ap()[hh * P:(hh + 1) * P, b * S:(b + 1) * S])
                        nc.sync.dma_start(
                            v_i,
                            vd.ap()[b * S:(b + 1) * S,
                                    hh * P:(hh + 1) * P]
                            .rearrange("(n p) e -> p n e", p=P))
                        for it in range(NB):
                            jlist = [j for j in range(NB)
                                     if kinds[it, j] != -1]
                            assert jlist, "fully-masked query row block"
                            pieces = _pieces(jlist)
                            npc = len(pieces)
                            prow = ppool.tile([P, S], BF, tag="prow")
                            den = dpool.tile([P, 16], F32, tag="den")
                            pos = 0
                            for pi, run in enumerate(pieces):
                                w = len(run) * P
                                j0 = run[0]
                                sps = spp.tile([P, 512], F32, tag="sps")
                                nc.tensor.matmul(
                                    sps[:, :w],
                                    qt_i[:, it * P:(it + 1) * P],
                                    kt_i[:, j0 * P:j0 * P + w],
                                    start=True, stop=True)
                                for bi, j in enumerate(run):
                                    k = kinds[it, j]
                                    if k >= 0:
                                        nc.vector.tensor_add(
                                            sps[:, bi * P:(bi + 1) * P],
                                            sps[:, bi * P:(bi + 1) * P],
                                            mb_sb[:, k, :])
                                nc.scalar.activation(
                                    prow[:, pos * P:pos * P + w],
                                    sps[:, :w],
                                    mybir.ActivationFunctionType.Exp,
                                    scale=SCALE,
                                    accum_out=den[:, pi:pi + 1])
                                pos += len(run)
                            dsum = dpool.tile([P, 1], F32, tag="dsum")
                            nc.vector.reduce_sum(dsum, den[:, :npc],
                                                 axis=mybir.AxisListType.X)
                            rec = dpool.tile([P, 1], F32, tag="rec")
                            nc.vector.reciprocal(rec, dsum)
                            nc.vector.tensor_scalar_mul(
                                prow[:, :pos * P], prow[:, :pos * P], rec)
                            ots = otp.tile([P, P], F32, tag="ots")
                            for bi, j in enumerate(jlist):
                                tps2 = ptp.tile([P, P], BF, tag="tps2")
                                nc.tensor.transpose(
                                    tps2, prow[:, bi * P:(bi + 1) * P], ident)
                                ptb = ptpool.tile([P, P], BF, tag="ptb")
                                nc.any.tensor_copy(ptb, tps2)
                                nc.tensor.matmul(
                                    ots, v_i[:, j], ptb,
                                    start=(bi == 0),
                                    stop=(bi == len(jlist) - 1))
                            otb = opool.tile([P, P], BF, tag="otb")
                            nc.any.tensor_copy(otb, ots)
                            nc.sync.dma_start(
                                cc_in.ap()[hh * P:(hh + 1) * P,
                                           b * S + it * P:
                                           b * S + (it + 1) * P], otb)

            # ---------------- AllGather + wo slice ----------------------
            nc.gpsimd.collective_compute(
                "AllGather", mybir.AluOpType.bypass,
                ins=[cc_in.ap().opt()],
                outs=[cc_out.ap().opt()],
                replica_groups=[list(range(NCORES))],
            )

            with (
                tc.tile_pool(name="wop", bufs=1) as wop,
                tc.tile_pool(name="ccp", bufs=2) as ccp,
                tc.tile_pool(name="obp", bufs=2) as obp,
                tc.tile_pool(name="wpp", bufs=2, space="PSUM") as wpp,
            ):
                wo_sb = wop.tile([P, ND, EL], BF, name="wo_sb")
                nc.sync.dma_start(
                    wo_sb, wot.ap().rearrange("(n p) e -> p n e", p=P))
                for g in range(NT):
                    cct = ccp.tile([P, ND, P], BF, tag="cct")
                    nc.sync.dma_start(
                        cct,
                        cc_out.ap()[:, g * P:(g + 1) * P]
                        .rearrange("(n p) t -> p n t", p=P))
                    ops = wpp.tile([P, EL], F32, tag="ops")
                    for ct in range(ND):
                        nc.tensor.matmul(ops, cct[:, ct], wo_sb[:, ct],
                                         start=(ct == 0),
                                         stop=(ct == ND - 1))
                    ob = obp.tile([P, EL], F32, tag="ob")
                    nc.any.tensor_copy(ob, ops)
                    nc.sync.dma_start(out.ap()[g * P:(g + 1) * P, :], ob)

    nc.compile()
    return nc


def kernel(x, wq, wk, wv, wo, freqs_cos, freqs_sin, mask, start_pos=0,
           **_ignored):
    global LAST_RESULTS
    bf = ml_dtypes.bfloat16
    mask = np.asarray(mask, np.float32)
    kinds, blocks = _classify_mask(mask)
    key = kinds.tobytes() + bytes([len(blocks)])
    nc = _GRAPH_CACHE.get(key)
    if nc is None:
        nc = _build(kinds, len(blocks))
        _GRAPH_CACHE[key] = nc

    xt_np = np.ascontiguousarray(
        np.asarray(x, np.float32).reshape(T, D).T).astype(bf)
    cos_r = np.ascontiguousarray(
        np.tile(np.asarray(freqs_cos, np.float32), (1, HLOC)))
    sin_r = np.ascontiguousarray(
        np.tile(np.asarray(freqs_sin, np.float32), (1, HLOC)))
    if blocks:
        mb_np = np.stack(blocks)
    else:
        mb_np = np.zeros((1, P, P), np.float32)

    in_maps = []
    for c in range(NCORES):
        hs = slice(c * HLOC, (c + 1) * HLOC)
        wq_c = np.ascontiguousarray(
            np.asarray(wq, np.float32)[hs].reshape(EL, D).T).astype(bf)
        wk_c = np.ascontiguousarray(
            np.asarray(wk, np.float32)[hs].reshape(EL, D).T).astype(bf)
        wv_c = np.ascontiguousarray(
            np.asarray(wv, np.float32)[hs].reshape(EL, D).T).astype(bf)
        wo_c = np.ascontiguousarray(
            np.asarray(wo, np.float32)[c * EL:(c + 1) * EL, :].T).astype(bf)
        in_maps.append({
            "xt": xt_np, "wqt": wq_c, "wkt": wk_c, "wvt": wv_c, "wot": wo_c,
            "cosr": cos_r, "sinr": sin_r, "mblk": mb_np,
        })

    res = run_bass_kernel_spmd(nc, in_maps, core_ids=list(range(NCORES)))
    LAST_RESULTS = res
    outs = [res.results[c]["out"] for c in range(NCORES)]
    full = np.concatenate(outs, axis=1).astype(np.float32)
    return full.reshape(B, S, D)


# revision 2
# speedup vs baseline: 1.0357x; 1.0357x over previous
"""v4: Tensor-parallel attention on 8 TRN2 cores.

Changes vs v3:
- Q/K written to DRAM in natural (token, feature) layout; attention loads
  Q^T/K^T via DMA xbar transpose (DRAM->SBUF), eliminating all PE transposes
  and their PSUM->SBUF copies in the projection.
- Weight and x tiles split into 8-d-tile sub-tiles so the first matmuls
  start after ~1.5MB of DMA instead of ~16MB.
- Freed PSUM banks -> den/ot pools double-buffered.


Changes vs v2:
- Per-batch interleaved build: proj(b0) -> attn(b0)+AG(b0) -> proj(b1) ->
  attn(b1)+AG(b1) -> wo.  Attention(b0) matmuls fill projection(b1) PE gaps,
  AllGather(b0) and wo(b0) overlap attention/projection of b1.
- PSUM split 4/4 between projection (one shared q/k/v tag ping-pong + 2
  transpose banks) and attention (2 score banks + den + ot) so both phases'
  pools coexist.
- Projection accumulates q/k/v sequentially per token tile (32 consecutive
  matmuls per group) instead of round-robin.
"""

import math
import sys

import numpy as np

sys.path.insert(0, "/opt/trn_rl_repo")

import ml_dtypes  # noqa: E402

import concourse.bass as bass  # noqa: E402,F401
import concourse.mybir as mybir  # noqa: E402
from concourse import bacc, tile  # noqa: E402
from concourse.bass_utils import run_bass_kernel_spmd  # noqa: E402
from concourse.masks import make_identity  # noqa: E402

B, S, D, H = 2, 2048, 4096, 32
HD = 128
NCORES = 8
HLOC = H // NCORES          # 4 heads per core
EL = HLOC * HD              # 512
T = B * S                   # 4096
P = 128
NT = T // P                 # 32
ND = D // P                 # 32
NB = S // P                 # 16 key blocks per batch
IC = 512                    # query-chunk width in attention
NIC = S // IC               # 4 query chunks per batch
SCALE = 1.0 / math.sqrt(HD)
BF = mybir.dt.bfloat16
F32 = mybir.dt.float32
TCH = 256
NCH_B = S // TCH            # x chunks per batch element (8)

_GRAPH_CACHE: dict[bytes, object] = {}
LAST_RESULTS = None


def _classify_mask(mask: np.ndarray):
    mb = mask.reshape(NB, P, NB, P).transpose(0, 2, 1, 3)
    kinds = np.empty((NB, NB), np.int64)
    uniq: dict[bytes, int] = {}
    blocks: list[np.ndarray] = []
    for i in range(NB):
        for j in range(NB):
            blk = mb[i, j]
            if not blk.any():
                kinds[i, j] = -2
            elif np.all(blk <= -1e8):
                kinds[i, j] = -1
            else:
                key = blk.tobytes()
                if key not in uniq:
                    uniq[key] = len(blocks)
                    blocks.append(np.ascontiguousarray(blk, np.float32))
                kinds[i, j] = uniq[key]
    return kinds, blocks


def _build(kinds: np.ndarray, n_blocks: int):
    nu = max(1, n_blocks)
    nc = bacc.Bacc("TRN2", target_bir_lowering=False, debug=False,
                   num_devices=NCORES)
    xt = nc.dram_tensor("xt", [D, T], BF, kind="ExternalInput")
    wqt = nc.dram_tensor("wqt", [D, EL], BF, kind="ExternalInput")
    wkt = nc.dram_tensor("wkt", [D, EL], BF, kind="ExternalInput")
    wvt = nc.dram_tensor("wvt", [D, EL], BF, kind="ExternalInput")
    wot = nc.dram_tensor("wot", [D, EL], BF, kind="ExternalInput")
    cosr = nc.dram_tensor("cosr", [S, EL // 2], F32, kind="ExternalInput")
    sinr = nc.dram_tensor("sinr", [S, EL // 2], F32, kind="ExternalInput")
    mblk = nc.dram_tensor("mblk", [nu, P, P], F32, kind="ExternalInput")
    out = nc.dram_tensor("out", [T, EL], F32, kind="ExternalOutput")

    qtd = [nc.dram_tensor(f"qnd{b}", [S, EL], BF) for b in range(B)]
    ktd = [nc.dram_tensor(f"knd{b}", [S, EL], BF) for b in range(B)]
    vd = [nc.dram_tensor(f"vd{b}", [S, EL], BF) for b in range(B)]
    NCC = B * NIC
    cc_in = [nc.dram_tensor(f"cc_in{k}", [EL, IC], BF) for k in range(NCC)]
    cc_out = [nc.dram_tensor(f"cc_out{k}", [NCORES * EL, IC], BF,
                             addr_space="Shared") for k in range(NCC)]

    jlists = []
    for icq in range(NIC):
        jl = [j for j in range(NB)
              if any(kinds[4 * icq + bi, j] != -1 for bi in range(4))]
        assert jl, "fully-masked query chunk"
        jlists.append(jl)

    with tile.TileContext(nc) as tc:
        with (
            tc.tile_pool(name="const", bufs=1) as cpool,
            tc.tile_pool(name="apool", bufs=2) as apool,
            tc.tile_pool(name="ptpool", bufs=4) as ptpool,
            tc.tile_pool(name="rpool", bufs=2) as rpool,
            tc.tile_pool(name="opool", bufs=2) as opool,
            tc.tile_pool(name="spp", bufs=2, space="PSUM") as spp,
            tc.tile_pool(name="dnp", bufs=2, space="PSUM") as dnp,
            tc.tile_pool(name="otp", bufs=2, space="PSUM") as otp,
        ):
            ones_sb = cpool.tile([P, P], BF, name="ones_sb")
            nc.vector.memset(ones_sb, 1.0)
            mb_sb = cpool.tile([P, nu, P], F32, name="mb_sb")
            nc.sync.dma_start(mb_sb, mblk.ap().rearrange("n p q -> p n q"))

            def proj_batch(b, w_parts, xpool, cspool, stg, rot, pp):
                for chb in range(NCH_B):
                    c0 = b * S + chb * TCH
                    xt_parts = []
                    for c4 in range(4):
                        xp = xpool.tile([P, 8, TCH], BF, tag=f"xt{c4}")
                        nc.sync.dma_start(
                            xp,
                            xt.ap()[c4 * 8 * P:(c4 + 1) * 8 * P,
                                    c0:c0 + TCH]
                            .rearrange("(n p) t -> p n t", p=P))
                        xt_parts.append(xp)
                    for tt in range(TCH // P):
                        s0 = chb * TCH + tt * P
                        cs_sb = cspool.tile([P, EL // 2], F32, tag="cs")
                        sn_sb = cspool.tile([P, EL // 2], F32, tag="sn")
                        nc.sync.dma_start(cs_sb, cosr.ap()[s0:s0 + P, :])
                        nc.sync.dma_start(sn_sb, sinr.ap()[s0:s0 + P, :])
                        for w_parts_i, dstd in ((0, None), (1, qtd),
                                                (2, ktd)):
                            wp = w_parts[w_parts_i]
                            ps = pp.tile([P, EL], F32, tag="pqkv")
                            for dt in range(ND):
                                nc.tensor.matmul(
                                    ps,
                                    xt_parts[dt // 8][:, dt % 8,
                                                      tt * P:(tt + 1) * P],
                                    wp[dt // 8][:, dt % 8],
                                    start=(dt == 0),
                                    stop=(dt == ND - 1))
                            if dstd is None:
                                v_stage = stg.tile([P, EL], BF, tag="vstg")
                                nc.any.tensor_copy(v_stage, ps)
                                nc.sync.dma_start(
                                    vd[b].ap()[s0:s0 + P, :], v_stage)
                                continue
                            qn = stg.tile([P, EL], BF, tag="qn")
                            pe = ps.rearrange("p (r two) -> p r two", two=2)
                            qe = qn.rearrange("p (r two) -> p r two", two=2)
                            t1 = rot.tile([P, EL // 2], F32, tag="t1")
                            t2 = rot.tile([P, EL // 2], F32, tag="t2")
                            nc.vector.tensor_mul(t1, pe[:, :, 0], cs_sb)
                            nc.vector.tensor_mul(t2, pe[:, :, 1], sn_sb)
                            nc.vector.tensor_sub(qe[:, :, 0], t1, t2)
                            nc.vector.tensor_mul(t1, pe[:, :, 0], sn_sb)
                            nc.vector.tensor_mul(t2, pe[:, :, 1], cs_sb)
                            nc.vector.tensor_add(qe[:, :, 1], t1, t2)
                            nc.sync.dma_start(
                                dstd[b].ap()[s0:s0 + P, :], qn)

            def attn_batch(b):
                for hh in range(HLOC):
                    qt_i = apool.tile([P, S], BF, tag="qt_i")
                    kt_i = apool.tile([P, S], BF, tag="kt_i")
                    v_i = apool.tile([P, NB, P], BF, tag="v_i")
                    nc.sync.dma_start_transpose(
                        qt_i, qtd[b].ap()[:, hh * P:(hh + 1) * P])
                    nc.sync.dma_start_transpose(
                        kt_i, ktd[b].ap()[:, hh * P:(hh + 1) * P])
                    nc.sync.dma_start(
                        v_i,
                        vd[b].ap()[:, hh * P:(hh + 1) * P]
                        .rearrange("(n p) e -> p n e", p=P))
                    for icq in range(NIC):
                        jl = jlists[icq]
                        den_ps = dnp.tile([P, IC], F32, tag="den_ps")
                        ot_ps = otp.tile([P, IC], F32, tag="ot_ps")
                        qslice = qt_i[:, icq * IC:(icq + 1) * IC]
                        for idx, j in enumerate(jl):
                            st = idx == 0
                            sp = idx == len(jl) - 1
                            sps = spp.tile([P, IC], F32, tag="sps")
                            nc.tensor.matmul(
                                sps, kt_i[:, j * P:(j + 1) * P], qslice,
                                start=True, stop=True)
                            for bi in range(4):
                                k = kinds[4 * icq + bi, j]
                                if k == -1:
                                    nc.vector.memset(
                                        sps[:, bi * P:(bi + 1) * P], -1e9)
                                elif k >= 0:
                                    nc.vector.tensor_add(
                                        sps[:, bi * P:(bi + 1) * P],
                                        sps[:, bi * P:(bi + 1) * P],
                                        mb_sb[:, k, :])
                            pt = ptpool.tile([P, IC], BF, tag="pt")
                            nc.scalar.activation(
                                pt, sps, mybir.ActivationFunctionType.Exp,
                                scale=SCALE)
                            nc.tensor.matmul(den_ps, ones_sb, pt,
                                             start=st, stop=sp)
                            nc.tensor.matmul(ot_ps, v_i[:, j], pt,
                                             start=st, stop=sp)
                        rec = rpool.tile([P, IC], F32, tag="rec")
                        nc.vector.reciprocal_approx_fast(rec, den_ps)
                        ot_sb = opool.tile([P, IC], BF, tag="ot_sb")
                        nc.vector.tensor_mul(ot_sb, ot_ps, rec)
                        nc.sync.dma_start(
                            cc_in[b * NIC + icq]
                            .ap()[hh * P:(hh + 1) * P, :], ot_sb)
                for icq in range(NIC):
                    k = b * NIC + icq
                    nc.gpsimd.collective_compute(
                        "AllGather", mybir.AluOpType.bypass,
                        ins=[cc_in[k].ap().opt()],
                        outs=[cc_out[k].ap().opt()],
                        replica_groups=[list(range(NCORES))],
                    )

            with (
                tc.tile_pool(name="wpool", bufs=1) as wpool,
                tc.tile_pool(name="xpool", bufs=2) as xpool,
                tc.tile_pool(name="cspool", bufs=2) as cspool,
                tc.tile_pool(name="stg", bufs=3) as stg,
                tc.tile_pool(name="rot", bufs=2) as rot,
                tc.tile_pool(name="pp", bufs=2, space="PSUM") as pp,
            ):
                w_parts = [[], [], []]
                for wi, w_d in ((0, wvt), (1, wqt), (2, wkt)):
                    for c4 in range(4):
                        wp = wpool.tile([P, 8, EL], BF,
                                        name=f"w{wi}_{c4}")
                        nc.sync.dma_start(
                            wp,
                            w_d.ap()[c4 * 8 * P:(c4 + 1) * 8 * P, :]
                            .rearrange("(n p) e -> p n e", p=P))
                        w_parts[wi].append(wp)
                proj_batch(0, w_parts, xpool, cspool, stg, rot, pp)
                attn_batch(0)
                proj_batch(1, w_parts, xpool, cspool, stg, rot, pp)
            attn_batch(1)

            with (
                tc.tile_pool(name="wop", bufs=1) as wop,
                tc.tile_pool(name="ccp", bufs=3) as ccp,
                tc.tile_pool(name="obp", bufs=2) as obp,
                tc.tile_pool(name="wpp", bufs=2, space="PSUM") as wpp,
            ):
                wo_sb = wop.tile([P, ND, EL], BF, name="wo_sb")
                nc.sync.dma_start(
                    wo_sb, wot.ap().rearrange("(n p) e -> p n e", p=P))
                for k in range(NCC):
                    for tt in range(IC // P):
                        g = k * (IC // P) + tt
                        cct = ccp.tile([P, ND, P], BF, tag="cct")
                        nc.sync.dma_start(
                            cct,
                            cc_out[k].ap()[:, tt * P:(tt + 1) * P]
                            .rearrange("(n p) t -> p n t", p=P))
                        ops = wpp.tile([P, EL], F32, tag="ops")
                        for ct in range(ND):
                            nc.tensor.matmul(ops, cct[:, ct], wo_sb[:, ct],
                                             start=(ct == 0),
                                             stop=(ct == ND - 1))
                        ob = obp.tile([P, EL], F32, tag="ob")
                        nc.any.tensor_copy(ob, ops)
                        nc.sync.dma_start(out.ap()[g * P:(g + 1) * P, :], ob)

    nc.compile()
    return nc


def kernel(x, wq, wk, wv, wo, freqs_cos, freqs_sin, mask, start_pos=0,
           **_ignored):
    global LAST_RESULTS
    bf = ml_dtypes.bfloat16
    mask = np.asarray(mask, np.float32)
    kinds, blocks = _classify_mask(mask)
    key = kinds.tobytes() + bytes([len(blocks)])
    nc = _GRAPH_CACHE.get(key)
    if nc is None:
        nc = _build(kinds, len(blocks))
        _GRAPH_CACHE[key] = nc

    xt_np = np.ascontiguousarray(
        np.asarray(x, np.float32).reshape(T, D).T).astype(bf)
    cos_r = np.ascontiguousarray(
        np.tile(np.asarray(freqs_cos, np.float32), (1, HLOC)))
    sin_r = np.ascontiguousarray(
        np.tile(np.asarray(freqs_sin, np.float32), (1, HLOC)))
    if blocks:
        mb_np = np.ascontiguousarray(
            np.stack([b.T for b in blocks]))  # transposed for ST layout
    else:
        mb_np = np.zeros((1, P, P), np.float32)

    in_maps = []
    for c in range(NCORES):
        hs = slice(c * HLOC, (c + 1) * HLOC)
        wq_c = np.ascontiguousarray(
            np.asarray(wq, np.float32)[hs].reshape(EL, D).T).astype(bf)
        wk_c = np.ascontiguousarray(
            np.asarray(wk, np.float32)[hs].reshape(EL, D).T).astype(bf)
        wv_c = np.ascontiguousarray(
            np.asarray(wv, np.float32)[hs].reshape(EL, D).T).astype(bf)
        wo_c = np.ascontiguousarray(
            np.asarray(wo, np.float32)[c * EL:(c + 1) * EL, :].T).astype(bf)
        in_maps.append({
            "xt": xt_np, "wqt": wq_c, "wkt": wk_c, "wvt": wv_c, "wot": wo_c,
            "cosr": cos_r, "sinr": sin_r, "mblk": mb_np,
        })

    res = run_bass_kernel_spmd(nc, in_maps, core_ids=list(range(NCORES)))
    LAST_RESULTS = res
    outs = [res.results[c]["out"] for c in range(NCORES)]
    full = np.concatenate(outs, axis=1).astype(np.float32)
    return full.reshape(B, S, D)


# revision 3
# speedup vs baseline: 1.0571x; 1.0207x over previous
"""v4: Tensor-parallel attention on 8 TRN2 cores.

Changes vs v3:
- Q/K written to DRAM in natural (token, feature) layout; attention loads
  Q^T/K^T via DMA xbar transpose (DRAM->SBUF), eliminating all PE transposes
  and their PSUM->SBUF copies in the projection.
- Weight and x tiles split into 8-d-tile sub-tiles so the first matmuls
  start after ~1.5MB of DMA instead of ~16MB.
- Freed PSUM banks -> den/ot pools double-buffered.


Changes vs v2:
- Per-batch interleaved build: proj(b0) -> attn(b0)+AG(b0) -> proj(b1) ->
  attn(b1)+AG(b1) -> wo.  Attention(b0) matmuls fill projection(b1) PE gaps,
  AllGather(b0) and wo(b0) overlap attention/projection of b1.
- PSUM split 4/4 between projection (one shared q/k/v tag ping-pong + 2
  transpose banks) and attention (2 score banks + den + ot) so both phases'
  pools coexist.
- Projection accumulates q/k/v sequentially per token tile (32 consecutive
  matmuls per group) instead of round-robin.
"""

import math
import sys

import numpy as np

sys.path.insert(0, "/opt/trn_rl_repo")

import ml_dtypes  # noqa: E402

import concourse.bass as bass  # noqa: E402,F401
import concourse.mybir as mybir  # noqa: E402
from concourse import bacc, tile  # noqa: E402
from concourse.bass_utils import run_bass_kernel_spmd  # noqa: E402
from concourse.masks import make_identity  # noqa: E402

B, S, D, H = 2, 2048, 4096, 32
HD = 128
NCORES = 8
HLOC = H // NCORES          # 4 heads per core
EL = HLOC * HD              # 512
T = B * S                   # 4096
P = 128
NT = T // P                 # 32
ND = D // P                 # 32
NB = S // P                 # 16 key blocks per batch
IC = 512                    # query-chunk width in attention
NIC = S // IC               # 4 query chunks per batch
SCALE = 1.0 / math.sqrt(HD)
BF = mybir.dt.bfloat16
F32 = mybir.dt.float32
TCH = 256
NCH_B = S // TCH            # x chunks per batch element (8)

_GRAPH_CACHE: dict[bytes, object] = {}
LAST_RESULTS = None


def _classify_mask(mask: np.ndarray):
    mb = mask.reshape(NB, P, NB, P).transpose(0, 2, 1, 3)
    kinds = np.empty((NB, NB), np.int64)
    uniq: dict[bytes, int] = {}
    blocks: list[np.ndarray] = []
    for i in range(NB):
        for j in range(NB):
            blk = mb[i, j]
            if not blk.any():
                kinds[i, j] = -2
            elif np.all(blk <= -1e8):
                kinds[i, j] = -1
            else:
                key = blk.tobytes()
                if key not in uniq:
                    uniq[key] = len(blocks)
                    blocks.append(np.ascontiguousarray(blk, np.float32))
                kinds[i, j] = uniq[key]
    return kinds, blocks


def _build(kinds: np.ndarray, n_blocks: int):
    nu = max(1, n_blocks)
    nc = bacc.Bacc("TRN2", target_bir_lowering=False, debug=False,
                   num_devices=NCORES)
    xt = nc.dram_tensor("xt", [D, T], BF, kind="ExternalInput")
    wqt = nc.dram_tensor("wqt", [D, EL], BF, kind="ExternalInput")
    wkt = nc.dram_tensor("wkt", [D, EL], BF, kind="ExternalInput")
    wvt = nc.dram_tensor("wvt", [D, EL], BF, kind="ExternalInput")
    wot = nc.dram_tensor("wot", [D, EL], BF, kind="ExternalInput")
    cosr = nc.dram_tensor("cosr", [S, EL // 2], F32, kind="ExternalInput")
    sinr = nc.dram_tensor("sinr", [S, EL // 2], F32, kind="ExternalInput")
    mblk = nc.dram_tensor("mblk", [nu, P, P], F32, kind="ExternalInput")
    out = nc.dram_tensor("out", [T, EL], F32, kind="ExternalOutput")

    qtd = [nc.dram_tensor(f"qnd{b}", [S, EL], BF) for b in range(B)]
    ktd = [nc.dram_tensor(f"knd{b}", [S, EL], BF) for b in range(B)]
    vd = [nc.dram_tensor(f"vd{b}", [S, EL], BF) for b in range(B)]
    NCC = B * NIC
    cc_in = [nc.dram_tensor(f"cc_in{k}", [EL, IC], BF) for k in range(NCC)]
    cc_out = [nc.dram_tensor(f"cc_out{k}", [NCORES * EL, IC], BF,
                             addr_space="Shared") for k in range(NCC)]

    jlists = []
    for icq in range(NIC):
        jl = [j for j in range(NB)
              if any(kinds[4 * icq + bi, j] != -1 for bi in range(4))]
        assert jl, "fully-masked query chunk"
        jlists.append(jl)

    with tile.TileContext(nc) as tc:
        with (
            tc.tile_pool(name="const", bufs=1) as cpool,
            tc.tile_pool(name="apool", bufs=2) as apool,
            tc.tile_pool(name="ptpool", bufs=4) as ptpool,
            tc.tile_pool(name="rpool", bufs=2) as rpool,
            tc.tile_pool(name="opool", bufs=2) as opool,
            tc.tile_pool(name="spp", bufs=2, space="PSUM") as spp,
            tc.tile_pool(name="dnp", bufs=2, space="PSUM") as dnp,
            tc.tile_pool(name="otp", bufs=2, space="PSUM") as otp,
        ):
            ones_sb = cpool.tile([P, P], BF, name="ones_sb")
            nc.vector.memset(ones_sb, 1.0)
            mb_sb = cpool.tile([P, nu, P], F32, name="mb_sb")
            nc.scalar.dma_start(mb_sb, mblk.ap().rearrange("n p q -> p n q"))

            def proj_batch(b, w_parts, xpool, cspool, stg, rot, pp):
                for chb in range(NCH_B):
                    c0 = b * S + chb * TCH
                    xt_parts = []
                    for c4 in range(4):
                        xp = xpool.tile([P, 8, TCH], BF, tag=f"xt{c4}")
                        nc.sync.dma_start(
                            xp,
                            xt.ap()[c4 * 8 * P:(c4 + 1) * 8 * P,
                                    c0:c0 + TCH]
                            .rearrange("(n p) t -> p n t", p=P))
                        xt_parts.append(xp)
                    for tt in range(TCH // P):
                        s0 = chb * TCH + tt * P
                        cs_sb = cspool.tile([P, EL // 2], F32, tag="cs")
                        sn_sb = cspool.tile([P, EL // 2], F32, tag="sn")
                        nc.sync.dma_start(cs_sb, cosr.ap()[s0:s0 + P, :])
                        nc.sync.dma_start(sn_sb, sinr.ap()[s0:s0 + P, :])
                        for w_parts_i, dstd in ((0, None), (1, qtd),
                                                (2, ktd)):
                            wp = w_parts[w_parts_i]
                            ps = pp.tile([P, EL], F32, tag="pqkv")
                            for dt in range(ND):
                                nc.tensor.matmul(
                                    ps,
                                    xt_parts[dt // 8][:, dt % 8,
                                                      tt * P:(tt + 1) * P],
                                    wp[dt // 8][:, dt % 8],
                                    start=(dt == 0),
                                    stop=(dt == ND - 1))
                            if dstd is None:
                                v_stage = stg.tile([P, EL], BF, tag="vstg")
                                nc.any.tensor_copy(v_stage, ps)
                                nc.sync.dma_start(
                                    vd[b].ap()[s0:s0 + P, :], v_stage)
                                continue
                            qn = stg.tile([P, EL], BF, tag="qn")
                            pe = ps.rearrange("p (r two) -> p r two", two=2)
                            qe = qn.rearrange("p (r two) -> p r two", two=2)
                            t1 = rot.tile([P, EL // 2], F32, tag="t1")
                            t2 = rot.tile([P, EL // 2], F32, tag="t2")
                            nc.vector.tensor_mul(t1, pe[:, :, 0], cs_sb)
                            nc.vector.tensor_mul(t2, pe[:, :, 1], sn_sb)
                            nc.vector.tensor_sub(qe[:, :, 0], t1, t2)
                            nc.vector.tensor_mul(t1, pe[:, :, 0], sn_sb)
                            nc.vector.tensor_mul(t2, pe[:, :, 1], cs_sb)
                            nc.vector.tensor_add(qe[:, :, 1], t1, t2)
                            nc.sync.dma_start(
                                dstd[b].ap()[s0:s0 + P, :], qn)

            def attn_batch(b):
                for hh in range(HLOC):
                    qt_i = apool.tile([P, S], BF, tag="qt_i")
                    kt_i = apool.tile([P, S], BF, tag="kt_i")
                    v_i = apool.tile([P, NB, P], BF, tag="v_i")
                    nc.sync.dma_start_transpose(
                        qt_i, qtd[b].ap()[:, hh * P:(hh + 1) * P])
                    nc.sync.dma_start_transpose(
                        kt_i, ktd[b].ap()[:, hh * P:(hh + 1) * P])
                    nc.sync.dma_start(
                        v_i,
                        vd[b].ap()[:, hh * P:(hh + 1) * P]
                        .rearrange("(n p) e -> p n e", p=P))
                    for icq in range(NIC):
                        jl = jlists[icq]
                        den_ps = dnp.tile([P, IC], F32, tag="den_ps")
                        ot_ps = otp.tile([P, IC], F32, tag="ot_ps")
                        qslice = qt_i[:, icq * IC:(icq + 1) * IC]
                        for idx, j in enumerate(jl):
                            st = idx == 0
                            sp = idx == len(jl) - 1
                            # leading fully-masked i-sub-blocks contribute 0:
                            # narrow all ops to the live suffix (first j in
                            # jl must be full-width to init the psum group)
                            nlead = 0
                            if not st:
                                for bi in range(4):
                                    if kinds[4 * icq + bi, j] == -1:
                                        nlead += 1
                                    else:
                                        break
                            off = nlead * P
                            w = IC - off
                            sps = spp.tile([P, IC], F32, tag="sps")
                            nc.tensor.matmul(
                                sps[:, off:], kt_i[:, j * P:(j + 1) * P],
                                qslice[:, off:], start=True, stop=True)
                            for bi in range(nlead, 4):
                                k = kinds[4 * icq + bi, j]
                                if k == -1:
                                    nc.vector.memset(
                                        sps[:, bi * P:(bi + 1) * P], -1e9)
                                elif k >= 0:
                                    nc.vector.tensor_add(
                                        sps[:, bi * P:(bi + 1) * P],
                                        sps[:, bi * P:(bi + 1) * P],
                                        mb_sb[:, k, :])
                            pt = ptpool.tile([P, IC], BF, tag="pt")
                            nc.scalar.activation(
                                pt[:, off:], sps[:, off:],
                                mybir.ActivationFunctionType.Exp,
                                scale=SCALE)
                            nc.tensor.matmul(den_ps[:, off:], ones_sb,
                                             pt[:, off:], start=st, stop=sp)
                            nc.tensor.matmul(ot_ps[:, off:], v_i[:, j],
                                             pt[:, off:], start=st, stop=sp)
                        rec = rpool.tile([P, IC], F32, tag="rec")
                        nc.vector.reciprocal_approx_fast(rec, den_ps)
                        ot_sb = opool.tile([P, IC], BF, tag="ot_sb")
                        nc.vector.tensor_mul(ot_sb, ot_ps, rec)
                        nc.sync.dma_start(
                            cc_in[b * NIC + icq]
                            .ap()[hh * P:(hh + 1) * P, :], ot_sb)
                for icq in range(NIC):
                    k = b * NIC + icq
                    nc.gpsimd.collective_compute(
                        "AllGather", mybir.AluOpType.bypass,
                        ins=[cc_in[k].ap().opt()],
                        outs=[cc_out[k].ap().opt()],
                        replica_groups=[list(range(NCORES))],
                    )

            with (
                tc.tile_pool(name="wpool", bufs=1) as wpool,
                tc.tile_pool(name="xpool", bufs=2) as xpool,
                tc.tile_pool(name="cspool", bufs=2) as cspool,
                tc.tile_pool(name="stg", bufs=3) as stg,
                tc.tile_pool(name="rot", bufs=2) as rot,
                tc.tile_pool(name="pp", bufs=2, space="PSUM") as pp,
            ):
                w_parts = [[], [], []]
                for wi, w_d in ((0, wvt), (1, wqt), (2, wkt)):
                    for c4 in range(4):
                        wp = wpool.tile([P, 8, EL], BF,
                                        name=f"w{wi}_{c4}")
                        eng = nc.sync if (wi == 0 and c4 == 0) else nc.scalar
                        eng.dma_start(
                            wp,
                            w_d.ap()[c4 * 8 * P:(c4 + 1) * 8 * P, :]
                            .rearrange("(n p) e -> p n e", p=P))
                        w_parts[wi].append(wp)
                proj_batch(0, w_parts, xpool, cspool, stg, rot, pp)
                attn_batch(0)
                proj_batch(1, w_parts, xpool, cspool, stg, rot, pp)
            attn_batch(1)

            with (
                tc.tile_pool(name="wop", bufs=1) as wop,
                tc.tile_pool(name="ccp", bufs=3) as ccp,
                tc.tile_pool(name="obp", bufs=2) as obp,
                tc.tile_pool(name="wpp", bufs=2, space="PSUM") as wpp,
            ):
                wo_sb = wop.tile([P, ND, EL], BF, name="wo_sb")
                nc.sync.dma_start(
                    wo_sb, wot.ap().rearrange("(n p) e -> p n e", p=P))
                for k in range(NCC):
                    for t2 in range(IC // (2 * P)):
                        cct = ccp.tile([P, ND, 2 * P], BF, tag="cct")
                        nc.sync.dma_start(
                            cct,
                            cc_out[k].ap()[:, t2 * 2 * P:(t2 + 1) * 2 * P]
                            .rearrange("(n p) t -> p n t", p=P))
                        for tt in range(2):
                            g = k * (IC // P) + t2 * 2 + tt
                            ops = wpp.tile([P, EL], F32, tag="ops")
                            for ct in range(ND):
                                nc.tensor.matmul(
                                    ops, cct[:, ct, tt * P:(tt + 1) * P],
                                    wo_sb[:, ct],
                                    start=(ct == 0), stop=(ct == ND - 1))
                            ob = obp.tile([P, EL], F32, tag="ob")
                            nc.any.tensor_copy(ob, ops)
                            nc.sync.dma_start(
                                out.ap()[g * P:(g + 1) * P, :], ob)

    nc.compile()
    return nc


def kernel(x, wq, wk, wv, wo, freqs_cos, freqs_sin, mask, start_pos=0,
           **_ignored):
    global LAST_RESULTS
    bf = ml_dtypes.bfloat16
    mask = np.asarray(mask, np.float32)
    kinds, blocks = _classify_mask(mask)
    key = kinds.tobytes() + bytes([len(blocks)])
    nc = _GRAPH_CACHE.get(key)
    if nc is None:
        nc = _build(kinds, len(blocks))
        _GRAPH_CACHE[key] = nc

    xt_np = np.ascontiguousarray(
        np.asarray(x, np.float32).reshape(T, D).T).astype(bf)
    cos_r = np.ascontiguousarray(
        np.tile(np.asarray(freqs_cos, np.float32), (1, HLOC)))
    sin_r = np.ascontiguousarray(
        np.tile(np.asarray(freqs_sin, np.float32), (1, HLOC)))
    if blocks:
        mb_np = np.ascontiguousarray(
            np.stack([b.T for b in blocks]))  # transposed for ST layout
    else:
        mb_np = np.zeros((1, P, P), np.float32)

    in_maps = []
    for c in range(NCORES):
        hs = slice(c * HLOC, (c + 1) * HLOC)
        wq_c = np.ascontiguousarray(
            np.asarray(wq, np.float32)[hs].reshape(EL, D).T).astype(bf)
        wk_c = np.ascontiguousarray(
            np.asarray(wk, np.float32)[hs].reshape(EL, D).T).astype(bf)
        wv_c = np.ascontiguousarray(
            np.asarray(wv, np.float32)[hs].reshape(EL, D).T).astype(bf)
        wo_c = np.ascontiguousarray(
            np.asarray(wo, np.float32)[c * EL:(c + 1) * EL, :].T).astype(bf)
        in_maps.append({
            "xt": xt_np, "wqt": wq_c, "wkt": wk_c, "wvt": wv_c, "wot": wo_c,
            "cosr": cos_r, "sinr": sin_r, "mblk": mb_np,
        })

    res = run_bass_kernel_spmd(nc, in_maps, core_ids=list(range(NCORES)))
    LAST_RESULTS = res
    outs = [res.results[c]["out"] for c in range(NCORES)]
    full = np.concatenate(outs, axis=1).astype(np.float32)
    return full.reshape(B, S, D)
